# revision 6
# baseline (speedup 1.0000x reference)
"""DeepGEMM-style fp8 linear on 8 TRN2 NeuronCores.

Computes: out = bf16( fp8(x_pad) @ (fp8(W) * block_scale).T ) + bias, sliced to
[16384, 4000], matching the jax reference (block scales are ones, bias zeros).

Strategy: batch-parallel SPMD with HOST-side fp8 quantization. Each core gets
a 2048-row batch shard of x, pre-quantized to fp8_e4m3 and transposed to
[k, b] on host, plus the full weight pre-quantized and transposed to [k, n]
blocks. fp8 quantization on host is bit-identical to the reference's
float8_e4m3fn round-trip for this value range (verified: e4m3 and e4m3fn
encodings coincide below the e4m3 max). On device: stream fp8 tiles, fp8
matmul with DoubleRow perf mode accumulating in fp32 PSUM, add bias + cast
to bf16, store out as [n, b]; host transposes/concats the shards back.

Why: with f32 inputs the kernel moves 117MB/core (DMA-floor ~330-350us);
with fp8 inputs it moves 42MB/core (~120us), making the kernel PE-bound.
HW-measured fp8 DoubleRow runs ~190ns per [128,512] matmul (0.85 cyc/col
incl. the per-matmul LDWEIGHTS, which walrus emits per instruction with
ldw-opt disabled) => ~390us of PE for the 2048-matmul schedule; the x-load
head bubble (~24us, DMA-paced) is absorbed by a ramp phase that computes
first-half-K partials for the first ramp_nt n-tiles while x streams in.
Measured 361-483us per exec depending on device contention (baseline 628us).
PSUM matmul output is ISA-capped at 512 f32 (one bank) => bg=512, 4 groups,
8-bank rotation. DoubleRowSwInterleave mis-computes with this weight layout
and is slower in-situ; plain DoubleRow is the right mode.
"""

import sys

if "/opt/trn_rl_repo" not in sys.path:
    sys.path.insert(0, "/opt/trn_rl_repo")

import numpy as np
import ml_dtypes

P = 128
N_CORES = 8
BATCH = 16384
IN_F = 4000
OUT_F = 4000
K_PAD = 4096               # in-features padded to 32 k-subtiles of 128
N_PAD = 4096               # out-features padded 4032 -> 4096 (uniform n-tiles)

_kernel_cache = {}

# test.py knobs
TRACE = False
LAST_RESULTS = None

# Bass lowers every matmul into a standalone InstLdweights + InstMatmult
# (ldweights=False) pair; a DoubleRow LDWEIGHTS streams 256 weight columns
# (~200ns), which matches or exceeds the ~107-213ns MM streaming time, so
# per-MM reloads of the SAME weight pair (our g-loop reuses each weight 4x)
# make the weight-load path the PE bottleneck. walrus --enable-ldw-opt
# rejects DoubleRow, so dedup at the BIR level instead: drop an
# InstLdweights when it is sync-free and reloads exactly what the previous
# (kept) LDW already put in the array. MMs never clobber array weights;
# any other PE-engine instruction resets the tracked state.
DEDUP_LDW = True


def _dedup_ldweights(nc):
    from concourse import mybir
    removed = kept = 0
    for b in nc.m.functions[0].blocks:
        keep = []
        last_key = None
        for i in b.instructions:
            if isinstance(i, mybir.InstLdweights):
                si = i.sync_info
                clean = si is None or (not si.on_wait and not si.on_update)
                key = (str(i.ins[0]), str(i.perf_mode), str(i.tile_position),
                       str(i.tile_size), i.is_transpose)
                if clean and key == last_key:
                    removed += 1
                    continue
                last_key = key
                kept += 1
            elif isinstance(i, mybir.InstMatmult):
                assert i.ldweights is False
                if i.is_transpose:        # transpose-mode clobbers the array
                    last_key = None
            elif getattr(i, "engine", None) == mybir.EngineType.PE:
                last_key = None
            keep.append(i)
        if removed:
            del b.instructions[:]
            b.instructions.extend(keep)
    return removed, kept


def _build(b_sh, ks, nt, bg, reps=1, xg=4, ramp_nt=3, wq_bufs=3, out_bufs=2,
           out_ring="sync", epi_split=False, probe="", unroll=1, pmode="dr"):
    """probe: '' normal kernel; 'pe' = x+w0 hoisted out of the reps loop,
    matmuls+epilogue only (pure PE rate); 'pe+w' = x hoisted, w streamed;
    'dma' = DMAs only, no compute (pure DMA rate).
    unroll: python-level body repetition (for TimelineSim, which cannot
    resolve For_i branches)."""
    import contextlib
    from concourse import bacc, tile, mybir
    from concourse.mybir import dt

    nbg = b_sh // bg
    assert nbg * bg == b_sh
    nxg = ks // xg
    assert nxg * xg == ks
    kk = ks // 2                      # DoubleRow k-pairs
    nc = bacc.Bacc(None, target_bir_lowering=False, debug=False)

    with tile.TileContext(nc) as tc:
        with tc.tile_pool(name="dram", bufs=1, space="DRAM") as dram:
            xt = dram.tile([nxg, P, xg, b_sh], dt.float8e4, kind="ExternalInput",
                           name="xt", uniquify=False)
            wp = dram.tile([nt, P, ks, P], dt.float8e4, kind="ExternalInput",
                           name="wp", uniquify=False)
            bvec = dram.tile([P, nt], dt.bfloat16, kind="ExternalInput",
                             name="bvec", uniquify=False)
            out = dram.tile([nt, P, b_sh], dt.bfloat16, kind="ExternalOutput",
                            name="out", uniquify=False)

        with tc.tile_pool(name="const", bufs=1) as const, \
             tc.tile_pool(name="xqp", bufs=(2 if unroll > 1 else 1)) as xqp, \
             tc.tile_pool(name="wqp", bufs=wq_bufs) as wqp, \
             tc.tile_pool(name="prtp", bufs=max(ramp_nt, 1)) as prtp, \
             tc.tile_pool(name="outp", bufs=out_bufs) as outp, \
             tc.tile_pool(name="psp",
                          bufs=max(2, min(8, (8 * 512) // max(bg, 512))),
                          space="PSUM") as psp:

            def load_bias():
                bias_bf = const.tile([P, nt], dt.bfloat16)
                nc.sync.dma_start(out=bias_bf[:, :], in_=bvec[:, :])
                bias_sb = const.tile([P, nt], dt.float32)
                nc.vector.tensor_copy(bias_sb[:, :], bias_bf[:, :])
                return bias_sb

            def load_x(xq):
                # stream fp8 straight into the resident tile, xg k-subtiles
                # per DMA (xg*b_sh contiguous bytes per partition)
                for g in range(nxg):
                    nc.sync.dma_start(out=xq[:, g * xg:(g + 1) * xg, :],
                                      in_=xt[g])

            def load_w(n):
                # weight n-tile: fp8 [P, ks, P] (4KB/partition contiguous) on
                # scalar's HWDGE ring so w loads don't queue behind x loads.
                wq = wqp.tile([P, ks, P], dt.float8e4, name="wq")
                nc.scalar.dma_start(out=wq[:, :, :], in_=wp[n])
                return wq

            xq = xqp.tile([P, ks, b_sh], dt.float8e4)

            pm = (mybir.MatmulPerfMode.DoubleRow if pmode == "dr"
                  else mybir.MatmulPerfMode.DoubleRowSwInterleave)

            def mm(wq, ps, g, k, start, stop):
                nc.tensor.matmul(
                    ps[:, :],
                    lhsT=wq[:, 2 * k:2 * k + 2, :],
                    rhs=xq[:, 2 * k:2 * k + 2, g * bg:(g + 1) * bg],
                    start=start, stop=stop,
                    perf_mode=pm)

            def ldw(wq, k):
                # explicit stationary load; probes whether walrus pairs it
                # with the following matmuls instead of re-loading per MM
                nc.tensor.ldweights(wq[:, 2 * k:2 * k + 2, :], perf_mode=pm)

            hoisted = probe in ("pe", "pe+w", "pe0", "peld")
            bias_sb = load_bias()
            wq0 = None
            if hoisted:
                load_x(xq)
                if probe in ("pe", "pe0", "peld"):
                    wq0 = load_w(0)

            with (tc.For_i(0, reps, 1) if reps > 1
                  else contextlib.nullcontext()):
              for _rep in range(unroll):
                if not hoisted:
                    load_x(xq)

                if probe == "dma":
                    # DMAs only: x (above) + w stream + out stores
                    junk = const.tile([P, b_sh], dt.bfloat16, name="junk")
                    nc.vector.memzero(junk[:, :])
                    for n in range(nt):
                        load_w(n)
                        ring = nc.sync if out_ring == "sync" else nc.scalar
                        ring.dma_start(out=out[n], in_=junk[:, :])
                else:
                    # Ramp phase: while x streams in, run first-half-K
                    # accumulation for the first ramp_nt n-tiles (uses only
                    # the first half of x); partials park in SBUF f32.
                    half = kk // 2
                    ramp_wq, ramp_part = {}, {}
                    for n in range(ramp_nt):
                        rwq = load_w(n)
                        ramp_wq[n] = rwq
                        part = prtp.tile([P, b_sh], dt.float32, name="part")
                        ramp_part[n] = part
                        pss = [psp.tile([P, bg], mybir.dt.float32, name="ps")
                               for _ in range(nbg)]
                        for k in range(half):
                            for g in range(nbg):
                                mm(rwq, pss[g], g, k, k == 0, k == half - 1)
                        for g in range(nbg):
                            nc.vector.tensor_copy(
                                part[:, g * bg:(g + 1) * bg], pss[g][:, :])

                    for n in range(nt):
                        ramp = n < ramp_nt
                        if probe in ("pe", "pe0", "peld"):
                            wq = wq0
                        elif ramp:
                            wq = ramp_wq[n]
                        else:
                            wq = load_w(n)

                        out_sb = outp.tile([P, b_sh], dt.bfloat16,
                                           name="out_sb")
                        k_lo = half if ramp else 0

                        pss = [psp.tile([P, bg], mybir.dt.float32, name="ps")
                               for _ in range(nbg)]
                        for k in range(k_lo, kk):
                            if probe == "peld":
                                ldw(wq, k)
                            for g in range(nbg):
                                mm(wq, pss[g], g, k, k == k_lo, k == kk - 1)
                        if probe == "pe0" and n != nt - 1:
                            continue
                        for g in range(nbg):
                            dst = out_sb[:, g * bg:(g + 1) * bg]
                            eng = nc.scalar if (epi_split and g % 2) \
                                else nc.vector
                            if ramp:
                                # (psum + bias) + first-half partial -> bf16
                                eng.scalar_tensor_tensor(
                                    dst, pss[g][:, :], bias_sb[:, n:n + 1],
                                    ramp_part[n][:, g * bg:(g + 1) * bg],
                                    mybir.AluOpType.add, mybir.AluOpType.add)
                            else:
                                eng.tensor_scalar_add(dst, pss[g][:, :],
                                                      bias_sb[:, n:n + 1])

                        if probe not in ("pe", "pe+w", "peld") or n == nt - 1:
                            ring = nc.sync if out_ring == "sync" \
                                else nc.scalar
                            ring.dma_start(out=out[n], in_=out_sb[:, :])

    nc.finalize()
    return nc


def make_key(reps=1):
    b_sh = BATCH // N_CORES
    return (b_sh, K_PAD // P, N_PAD // P, 512, reps)


def _get_nc(key):
    if key not in _kernel_cache:
        nc = _build(*key)
        if DEDUP_LDW:
            _dedup_ldweights(nc)
        _kernel_cache[key] = nc
    return _kernel_cache[key]


def kernel(x, weight, weight_scale, bias):
    global LAST_RESULTS
    from concourse.bass_utils import run_bass_kernel_spmd

    x = np.asarray(x, dtype=np.float32)
    weight = np.asarray(weight, dtype=np.float32)
    weight_scale = np.asarray(weight_scale, dtype=np.float32)
    bias = np.asarray(bias)  # bf16

    n_out, k_pad = weight.shape          # 4032, 4096
    batch, in_f = x.shape                # 16384, 4000
    assert k_pad == K_PAD and batch == BATCH

    b_sh = batch // N_CORES
    ks = K_PAD // P
    nt = N_PAD // P
    xg = 4
    f8 = ml_dtypes.float8_e4m3

    # Quantize weight on host, exactly as the reference does; fold non-one
    # block scales in post-quantization (exact for power-of-two scales).
    wq8 = weight.astype(ml_dtypes.float8_e4m3fn)
    if not np.allclose(weight_scale, 1.0):
        ws = np.repeat(np.repeat(weight_scale, P, axis=0), P, axis=1)
        wq8 = (wq8.astype(np.float32) * ws[:n_out, :k_pad]).astype(
            ml_dtypes.float8_e4m3fn)
    wpad = np.zeros((N_PAD, K_PAD), dtype=f8)
    wpad[:n_out] = wq8.view(np.uint8).view(f8)
    # w -> [nt, p, ks, j]: element = w[nt*128 + j, ks*128 + p], zero-pad rows
    wp = np.ascontiguousarray(wpad.reshape(nt, P, ks, P).transpose(0, 3, 2, 1))

    # x: quantize once, pad features to K_PAD
    xq8 = np.zeros((batch, K_PAD), dtype=f8)
    xq8[:, :in_f] = x.astype(ml_dtypes.float8_e4m3fn).view(np.uint8).view(f8)

    # bias -> [p, nt] bf16, zero-padded
    bpad = np.zeros(N_PAD, dtype=ml_dtypes.bfloat16)
    bpad[:n_out] = bias
    bvec = np.ascontiguousarray(bpad.reshape(nt, P).T)

    in_maps = []
    for c in range(N_CORES):
        shard = xq8[c * b_sh:(c + 1) * b_sh]        # [b_sh, K_PAD] fp8
        # -> [nxg, P, xg, b_sh]: element (g,p,j,b) = x[b, (g*xg+j)*P + p]
        xt = np.ascontiguousarray(
            shard.T.reshape(ks // xg, xg, P, b_sh).transpose(0, 2, 1, 3))
        in_maps.append({"xt": xt, "wp": wp, "bvec": bvec})

    global _last_in_maps
    _last_in_maps = in_maps
    nc = _get_nc(make_key(1))
    res = run_bass_kernel_spmd(nc, in_maps, list(range(N_CORES)), trace=TRACE)
    LAST_RESULTS = res

    final = np.empty((batch, OUT_F), dtype=ml_dtypes.bfloat16)
    for c in range(N_CORES):
        oc = res.results[c]["out"].reshape(N_PAD, b_sh)
        final[c * b_sh:(c + 1) * b_sh, :] = oc[:OUT_F].T
    return final



# revision 24
# speedup vs baseline: 1.3256x; 1.3256x over previous
"""DeepGEMM-style fp8 linear on 8 TRN2 NeuronCores.

Computes: out = bf16( fp8(x_pad) @ (fp8(W) * block_scale).T ) + bias, sliced to
[16384, 4000], matching the jax reference (block scales are ones, bias zeros).

Strategy: batch-parallel SPMD with HOST-side fp8 quantization. Each core gets
a 2048-row batch shard of x, pre-quantized to fp8_e4m3 and transposed to
[k, b] on host, plus the full weight pre-quantized and transposed to [k, n]
blocks. fp8 quantization on host is bit-identical to the reference's
float8_e4m3fn round-trip for this value range (verified: e4m3 and e4m3fn
encodings coincide below the e4m3 max). On device: stream fp8 tiles, fp8
matmul with DoubleRow perf mode accumulating in fp32 PSUM, add bias + cast
to bf16, store out as [n, b]; host transposes/concats the shards back.

Why: with f32 inputs the kernel moves 117MB/core (DMA-floor ~330-350us);
with fp8 inputs it moves 42MB/core (~120us), making the kernel PE-bound.
HW-measured fp8 DoubleRow runs ~190ns per [128,512] matmul (0.85 cyc/col
incl. the per-matmul LDWEIGHTS, which walrus emits per instruction with
ldw-opt disabled) => ~390us of PE for the 2048-matmul schedule; the x-load
head bubble (~24us, DMA-paced) is absorbed by a ramp phase that computes
first-half-K partials for the first ramp_nt n-tiles while x streams in.
Measured 361-483us per exec depending on device contention (baseline 628us).
PSUM matmul output is ISA-capped at 512 f32 (one bank) => bg=512, 4 groups,
8-bank rotation. DoubleRowSwInterleave mis-computes with this weight layout
and is slower in-situ; plain DoubleRow is the right mode.
"""

import sys

if "/opt/trn_rl_repo" not in sys.path:
    sys.path.insert(0, "/opt/trn_rl_repo")

import numpy as np
import ml_dtypes

P = 128
N_CORES = 8
BATCH = 16384
IN_F = 4000
OUT_F = 4000
K_PAD = 4096               # in-features padded to 32 k-subtiles of 128
N_PAD = 4096               # out-features padded 4032 -> 4096 (uniform n-tiles)

_kernel_cache = {}

# test.py knobs
TRACE = False
LAST_RESULTS = None

# Bass lowers every matmul into a standalone InstLdweights + InstMatmult
# (ldweights=False) pair; a DoubleRow LDWEIGHTS streams 256 weight columns
# (~200ns), which matches or exceeds the ~107-213ns MM streaming time, so
# per-MM reloads of the SAME weight pair (our g-loop reuses each weight 4x)
# make the weight-load path the PE bottleneck. walrus --enable-ldw-opt
# rejects DoubleRow, so dedup at the BIR level instead: drop an
# InstLdweights when it is sync-free and reloads exactly what the previous
# (kept) LDW already put in the array. MMs never clobber array weights;
# any other PE-engine instruction resets the tracked state.
DEDUP_LDW = True


def _dedup_ldweights(nc):
    from concourse import mybir
    removed = kept = 0
    for b in nc.m.functions[0].blocks:
        keep = []
        last_key = None
        for i in b.instructions:
            if isinstance(i, mybir.InstLdweights):
                si = i.sync_info
                clean = si is None or (not si.on_wait and not si.on_update)
                key = (str(i.ins[0]), str(i.perf_mode), str(i.tile_position),
                       str(i.tile_size), i.is_transpose)
                if clean and key == last_key:
                    removed += 1
                    continue
                last_key = key
                kept += 1
            elif isinstance(i, mybir.InstMatmult):
                assert i.ldweights is False
                if i.is_transpose:        # transpose-mode clobbers the array
                    last_key = None
            elif getattr(i, "engine", None) == mybir.EngineType.PE:
                last_key = None
            keep.append(i)
        if removed:
            del b.instructions[:]
            b.instructions.extend(keep)
    return removed, kept


def _build(b_sh, ks, nt, bg, reps=1, xg=4, ramp_nt=4, wq_bufs=6, out_bufs=2,
           out_ring="sync", epi_split=True, probe="", unroll=1, pmode="dr",
           x_two_rings=True, tail_split=True):
    """probe: '' normal kernel; 'pe' = x+w0 hoisted out of the reps loop,
    matmuls+epilogue only (pure PE rate); 'pe+w' = x hoisted, w streamed;
    'dma' = DMAs only, no compute (pure DMA rate).
    unroll: python-level body repetition (for TimelineSim, which cannot
    resolve For_i branches)."""
    import contextlib
    from concourse import bacc, tile, mybir
    from concourse.mybir import dt

    nbg = b_sh // bg
    assert nbg * bg == b_sh
    nxg = ks // xg
    assert nxg * xg == ks
    kk = ks // 2                      # DoubleRow k-pairs
    nc = bacc.Bacc(None, target_bir_lowering=False, debug=False)

    with tile.TileContext(nc) as tc:
        with tc.tile_pool(name="dram", bufs=1, space="DRAM") as dram:
            xt = dram.tile([nxg, P, xg, b_sh], dt.float8e4, kind="ExternalInput",
                           name="xt", uniquify=False)
            wp = dram.tile([nt, P, ks, P], dt.float8e4, kind="ExternalInput",
                           name="wp", uniquify=False)
            bvec = dram.tile([P, nt], dt.bfloat16, kind="ExternalInput",
                             name="bvec", uniquify=False)
            out = dram.tile([nt, P, b_sh], dt.bfloat16, kind="ExternalOutput",
                            name="out", uniquify=False)

        with tc.tile_pool(name="const", bufs=1) as const, \
             tc.tile_pool(name="xqp",
                          bufs=(2 if (unroll > 1 or reps > 1) else 1)) as xqp, \
             tc.tile_pool(name="wqp", bufs=wq_bufs) as wqp, \
             tc.tile_pool(name="prtp", bufs=max(ramp_nt, 1)) as prtp, \
             tc.tile_pool(name="outp", bufs=out_bufs) as outp, \
             tc.tile_pool(name="psp",
                          bufs=max(2, min(8, (8 * 512) // max(bg, 512))),
                          space="PSUM") as psp:

            def load_bias():
                bias_bf = const.tile([P, nt], dt.bfloat16)
                nc.sync.dma_start(out=bias_bf[:, :], in_=bvec[:, :])
                bias_sb = const.tile([P, nt], dt.float32)
                nc.vector.tensor_copy(bias_sb[:, :], bias_bf[:, :])
                return bias_sb

            def load_x(xq):
                # stream fp8 straight into the resident tile, xg k-subtiles
                # per DMA (xg*b_sh contiguous bytes per partition). One DMA
                # ring can't keep up with the ramp's k-loop (3.2us/group DMA
                # vs 1.7us/2-k-pairs of MM), so alternate groups across the
                # SP and Activation rings; group 0 is split in half so the
                # first MMs start after only xg/2 subtiles land.
                if not x_two_rings:
                    for g in range(nxg):
                        nc.sync.dma_start(out=xq[:, g * xg:(g + 1) * xg, :],
                                          in_=xt[g])
                    return
                h = xg // 2
                nc.sync.dma_start(out=xq[:, 0:h, :], in_=xt[0][:, 0:h, :])
                nc.scalar.dma_start(out=xq[:, h:xg, :], in_=xt[0][:, h:xg, :])
                for g in range(1, nxg):
                    ring = nc.sync if g % 2 == 0 else nc.scalar
                    ring.dma_start(out=xq[:, g * xg:(g + 1) * xg, :],
                                   in_=xt[g])

            def load_w(n):
                # weight n-tile: fp8 [P, ks, P] (4KB/partition contiguous) on
                # scalar's HWDGE ring so w loads don't queue behind x loads.
                wq = wqp.tile([P, ks, P], dt.float8e4, name="wq")
                nc.scalar.dma_start(out=wq[:, :, :], in_=wp[n])
                return wq

            # xq is re-allocated per reps-iteration from a 2-buf pool so
            # iteration i+1's x stream prefetches under iteration i's MMs
            # (no per-iteration head bubble in the repeat-timing NEFF).
            _xq_hold = [None]

            def new_xq():
                _xq_hold[0] = xqp.tile([P, ks, b_sh], dt.float8e4, name="xq")
                return _xq_hold[0]

            pm = (mybir.MatmulPerfMode.DoubleRow if pmode == "dr"
                  else mybir.MatmulPerfMode.DoubleRowSwInterleave)

            def mm(wq, ps, g, k, start, stop):
                xq = _xq_hold[0]
                nc.tensor.matmul(
                    ps[:, :],
                    lhsT=wq[:, 2 * k:2 * k + 2, :],
                    rhs=xq[:, 2 * k:2 * k + 2, g * bg:(g + 1) * bg],
                    start=start, stop=stop,
                    perf_mode=pm)

            def ldw(wq, k):
                # explicit stationary load; probes whether walrus pairs it
                # with the following matmuls instead of re-loading per MM
                nc.tensor.ldweights(wq[:, 2 * k:2 * k + 2, :], perf_mode=pm)

            hoisted = probe in ("pe", "pe+w", "pe0", "peld")
            bias_sb = load_bias()
            wq0 = None
            if hoisted:
                load_x(xq)
                if probe in ("pe", "pe0", "peld"):
                    wq0 = load_w(0)

            with (tc.For_i(0, reps, 1) if reps > 1
                  else contextlib.nullcontext()):
              for _rep in range(unroll):
                if not hoisted and probe == "dma":
                    load_x(xq)

                if probe == "dma":
                    # DMAs only: x (above) + w stream + out stores
                    junk = const.tile([P, b_sh], dt.bfloat16, name="junk")
                    nc.vector.memzero(junk[:, :])
                    for n in range(nt):
                        load_w(n)
                        ring = nc.sync if out_ring == "sync" else nc.scalar
                        ring.dma_start(out=out[n], in_=junk[:, :])
                else:
                    # Ramp phase: while x streams in, run first-half-K
                    # accumulation for the first ramp_nt n-tiles (uses only
                    # the first half of x); partials park in SBUF f32.
                    # Tile-serial with k inner; the 8 PSUM banks keep two
                    # tiles in flight so tile n+1's MMs overlap tile n's
                    # park copies. Emission order matters for ring FIFO: w0
                    # first (needed by the first LDW), then the x stream
                    # with the remaining ramp weights interleaved.
                    half = kk // 2
                    ramp_wq, ramp_part = {}, {}
                    ramp_wq[0] = load_w(0)
                    wi = 1
                    if not hoisted:
                        if x_two_rings:
                            # interleave the remaining ramp weights between
                            # the odd (Activation-ring) x groups so each w_n
                            # lands just before tile n's k-loop needs it
                            h = xg // 2
                            nc.sync.dma_start(out=xq[:, 0:h, :],
                                              in_=xt[0][:, 0:h, :])
                            nc.scalar.dma_start(out=xq[:, h:xg, :],
                                                in_=xt[0][:, h:xg, :])
                            for g in range(1, nxg):
                                ring = nc.sync if g % 2 == 0 else nc.scalar
                                ring.dma_start(
                                    out=xq[:, g * xg:(g + 1) * xg, :],
                                    in_=xt[g])
                                if g % 2 == 1 and wi < ramp_nt:
                                    ramp_wq[wi] = load_w(wi)
                                    wi += 1
                        else:
                            load_x(xq)
                    while wi < ramp_nt:
                        ramp_wq[wi] = load_w(wi)
                        wi += 1
                    for n in range(ramp_nt):
                        ramp_part[n] = prtp.tile([P, b_sh], dt.float32,
                                                 name="part")
                        pss = [psp.tile([P, bg], mybir.dt.float32, name="ps")
                               for _ in range(nbg)]
                        for k in range(half):
                            for g in range(nbg):
                                mm(ramp_wq[n], pss[g], g, k,
                                   k == 0, k == half - 1)
                        for g in range(nbg):
                            dst = ramp_part[n][:, g * bg:(g + 1) * bg]
                            if epi_split and g % 2:
                                nc.scalar.copy(dst, pss[g][:, :])
                            else:
                                nc.vector.tensor_copy(dst, pss[g][:, :])

                    for n in range(nt):
                        ramp = n < ramp_nt
                        if probe in ("pe", "pe0", "peld"):
                            wq = wq0
                        elif ramp:
                            wq = ramp_wq[n]
                        else:
                            wq = load_w(n)

                        out_sb = outp.tile([P, b_sh], dt.bfloat16,
                                           name="out_sb")
                        k_lo = half if ramp else 0
                        last = n == nt - 1
                        ring = nc.sync if out_ring == "sync" else nc.scalar

                        def epi(g, ps, store):
                            dst = out_sb[:, g * bg:(g + 1) * bg]
                            if ramp:
                                # (psum + bias) + first-half partial -> bf16
                                # (DVE only; Activation has no 3-operand op)
                                nc.vector.scalar_tensor_tensor(
                                    dst, ps[:, :], bias_sb[:, n:n + 1],
                                    ramp_part[n][:, g * bg:(g + 1) * bg],
                                    mybir.AluOpType.add, mybir.AluOpType.add)
                            elif epi_split and g % 2:
                                nc.scalar.add(dst, ps[:, :],
                                              bias_sb[:, n:n + 1])
                            else:
                                nc.vector.tensor_scalar_add(
                                    dst, ps[:, :], bias_sb[:, n:n + 1])
                            if store:
                                ring.dma_start(
                                    out=out[n][:, g * bg:(g + 1) * bg],
                                    in_=dst)

                        if last and tail_split and probe == "":
                            # Final n-tile: g OUTER, k inner, so each batch
                            # group's epilogue + store chunk pipelines under
                            # the next group's MMs; the kernel tail is one
                            # group's epilogue + one 128KB store instead of
                            # the whole tile's.
                            for g in range(nbg):
                                ps = psp.tile([P, bg], mybir.dt.float32,
                                              name="ps")
                                for k in range(k_lo, kk):
                                    mm(wq, ps, g, k, k == k_lo, k == kk - 1)
                                epi(g, ps, True)
                            continue

                        pss = [psp.tile([P, bg], mybir.dt.float32, name="ps")
                               for _ in range(nbg)]
                        for k in range(k_lo, kk):
                            if probe == "peld":
                                ldw(wq, k)
                            for g in range(nbg):
                                mm(wq, pss[g], g, k, k == k_lo, k == kk - 1)
                        if probe == "pe0" and n != nt - 1:
                            continue
                        for g in range(nbg):
                            epi(g, pss[g], False)
                        if probe not in ("pe", "pe+w", "peld") or last:
                            ring.dma_start(out=out[n], in_=out_sb[:, :])

    nc.finalize()
    return nc


def make_key(reps=1):
    b_sh = BATCH // N_CORES
    return (b_sh, K_PAD // P, N_PAD // P, 512, reps)


def _get_nc(key):
    if key not in _kernel_cache:
        nc = _build(*key)
        if DEDUP_LDW:
            _dedup_ldweights(nc)
        _kernel_cache[key] = nc
    return _kernel_cache[key]


def kernel(x, weight, weight_scale, bias):
    global LAST_RESULTS
    from concourse.bass_utils import run_bass_kernel_spmd

    x = np.asarray(x, dtype=np.float32)
    weight = np.asarray(weight, dtype=np.float32)
    weight_scale = np.asarray(weight_scale, dtype=np.float32)
    bias = np.asarray(bias)  # bf16

    n_out, k_pad = weight.shape          # 4032, 4096
    batch, in_f = x.shape                # 16384, 4000
    assert k_pad == K_PAD and batch == BATCH

    b_sh = batch // N_CORES
    ks = K_PAD // P
    nt = N_PAD // P
    xg = 4
    f8 = ml_dtypes.float8_e4m3

    # Quantize weight on host, exactly as the reference does; fold non-one
    # block scales in post-quantization (exact for power-of-two scales).
    wq8 = weight.astype(ml_dtypes.float8_e4m3fn)
    if not np.allclose(weight_scale, 1.0):
        ws = np.repeat(np.repeat(weight_scale, P, axis=0), P, axis=1)
        wq8 = (wq8.astype(np.float32) * ws[:n_out, :k_pad]).astype(
            ml_dtypes.float8_e4m3fn)
    wpad = np.zeros((N_PAD, K_PAD), dtype=f8)
    wpad[:n_out] = wq8.view(np.uint8).view(f8)
    # w -> [nt, p, ks, j]: element = w[nt*128 + j, ks*128 + p], zero-pad rows
    wp = np.ascontiguousarray(wpad.reshape(nt, P, ks, P).transpose(0, 3, 2, 1))

    # x: quantize once, pad features to K_PAD
    xq8 = np.zeros((batch, K_PAD), dtype=f8)
    xq8[:, :in_f] = x.astype(ml_dtypes.float8_e4m3fn).view(np.uint8).view(f8)

    # bias -> [p, nt] bf16, zero-padded
    bpad = np.zeros(N_PAD, dtype=ml_dtypes.bfloat16)
    bpad[:n_out] = bias
    bvec = np.ascontiguousarray(bpad.reshape(nt, P).T)

    in_maps = []
    for c in range(N_CORES):
        shard = xq8[c * b_sh:(c + 1) * b_sh]        # [b_sh, K_PAD] fp8
        # -> [nxg, P, xg, b_sh]: element (g,p,j,b) = x[b, (g*xg+j)*P + p]
        xt = np.ascontiguousarray(
            shard.T.reshape(ks // xg, xg, P, b_sh).transpose(0, 2, 1, 3))
        in_maps.append({"xt": xt, "wp": wp, "bvec": bvec})

    global _last_in_maps
    _last_in_maps = in_maps
    nc = _get_nc(make_key(1))
    res = run_bass_kernel_spmd(nc, in_maps, list(range(N_CORES)), trace=TRACE)
    LAST_RESULTS = res

    final = np.empty((batch, OUT_F), dtype=ml_dtypes.bfloat16)
    for c in range(N_CORES):
        oc = res.results[c]["out"].reshape(N_PAD, b_sh)
        final[c * b_sh:(c + 1) * b_sh, :] = oc[:OUT_F].T
    return final



# revision 26
# speedup vs baseline: 1.3916x; 1.0498x over previous
"""DeepGEMM-style fp8 linear on 8 TRN2 NeuronCores.

Computes: out = bf16( fp8(x_pad) @ (fp8(W) * block_scale).T ) + bias, sliced to
[16384, 4000], matching the jax reference (block scales are ones, bias zeros).

Strategy: batch-parallel SPMD with HOST-side fp8 quantization. Each core gets
a 2048-row batch shard of x, pre-quantized to fp8_e4m3 and transposed to
[k, b] on host, plus the full weight pre-quantized and transposed to [k, n]
blocks. fp8 quantization on host is bit-identical to the reference's
float8_e4m3fn round-trip for this value range (verified: e4m3 and e4m3fn
encodings coincide below the e4m3 max). On device: stream fp8 tiles, fp8
matmul with DoubleRow perf mode accumulating in fp32 PSUM, add bias + cast
to bf16, store out as [n, b]; host transposes/concats the shards back.

Why: with f32 inputs the kernel moves 117MB/core (DMA-floor ~330-350us);
with fp8 inputs it moves 42MB/core (~120us), making the kernel PE-bound.
HW-measured fp8 DoubleRow runs ~190ns per [128,512] matmul (0.85 cyc/col
incl. the per-matmul LDWEIGHTS, which walrus emits per instruction with
ldw-opt disabled) => ~390us of PE for the 2048-matmul schedule; the x-load
head bubble (~24us, DMA-paced) is absorbed by a ramp phase that computes
first-half-K partials for the first ramp_nt n-tiles while x streams in.
Measured 361-483us per exec depending on device contention (baseline 628us).
PSUM matmul output is ISA-capped at 512 f32 (one bank) => bg=512, 4 groups,
8-bank rotation. DoubleRowSwInterleave mis-computes with this weight layout
and is slower in-situ; plain DoubleRow is the right mode.
"""

import sys

if "/opt/trn_rl_repo" not in sys.path:
    sys.path.insert(0, "/opt/trn_rl_repo")

import numpy as np
import ml_dtypes

P = 128
N_CORES = 8
BATCH = 16384
IN_F = 4000
OUT_F = 4000
K_PAD = 4096               # in-features padded to 32 k-subtiles of 128
N_PAD = 4096               # out-features padded 4032 -> 4096 (uniform n-tiles)

_kernel_cache = {}

# test.py knobs
TRACE = False
LAST_RESULTS = None

# Bass lowers every matmul into a standalone InstLdweights + InstMatmult
# (ldweights=False) pair; a DoubleRow LDWEIGHTS streams 256 weight columns
# (~200ns), which matches or exceeds the ~107-213ns MM streaming time, so
# per-MM reloads of the SAME weight pair (our g-loop reuses each weight 4x)
# make the weight-load path the PE bottleneck. walrus --enable-ldw-opt
# rejects DoubleRow, so dedup at the BIR level instead: drop an
# InstLdweights when it is sync-free and reloads exactly what the previous
# (kept) LDW already put in the array. MMs never clobber array weights;
# any other PE-engine instruction resets the tracked state.
DEDUP_LDW = True


def _dedup_ldweights(nc):
    from concourse import mybir
    removed = kept = 0
    for b in nc.m.functions[0].blocks:
        keep = []
        last_key = None
        for i in b.instructions:
            if isinstance(i, mybir.InstLdweights):
                si = i.sync_info
                clean = si is None or (not si.on_wait and not si.on_update)
                key = (str(i.ins[0]), str(i.perf_mode), str(i.tile_position),
                       str(i.tile_size), i.is_transpose)
                if clean and key == last_key:
                    removed += 1
                    continue
                last_key = key
                kept += 1
            elif isinstance(i, mybir.InstMatmult):
                assert i.ldweights is False
                if i.is_transpose:        # transpose-mode clobbers the array
                    last_key = None
            elif getattr(i, "engine", None) == mybir.EngineType.PE:
                last_key = None
            keep.append(i)
        if removed:
            del b.instructions[:]
            b.instructions.extend(keep)
    return removed, kept


def _build(b_sh, ks, nt, bg, reps=1, xg=4, ramp_nt=4, wq_bufs=6, out_bufs=2,
           out_ring="sync", epi_split=True, probe="", unroll=1, pmode="dr",
           x_two_rings=True, tail_split=True):
    """probe: '' normal kernel; 'pe' = x+w0 hoisted out of the reps loop,
    matmuls+epilogue only (pure PE rate); 'pe+w' = x hoisted, w streamed;
    'dma' = DMAs only, no compute (pure DMA rate).
    unroll: python-level body repetition (for TimelineSim, which cannot
    resolve For_i branches)."""
    import contextlib
    from concourse import bacc, tile, mybir
    from concourse.mybir import dt

    nbg = b_sh // bg
    assert nbg * bg == b_sh
    nxg = ks // xg
    assert nxg * xg == ks
    kk = ks // 2                      # DoubleRow k-pairs
    nc = bacc.Bacc(None, target_bir_lowering=False, debug=False)

    with tile.TileContext(nc) as tc:
        with tc.tile_pool(name="dram", bufs=1, space="DRAM") as dram:
            xt = dram.tile([nxg, P, xg, b_sh], dt.float8e4, kind="ExternalInput",
                           name="xt", uniquify=False)
            wp = dram.tile([nt, P, ks, P], dt.float8e4, kind="ExternalInput",
                           name="wp", uniquify=False)
            bvec = dram.tile([P, nt], dt.bfloat16, kind="ExternalInput",
                             name="bvec", uniquify=False)
            out = dram.tile([nt, P, b_sh], dt.bfloat16, kind="ExternalOutput",
                            name="out", uniquify=False)

        with tc.tile_pool(name="const", bufs=1) as const, \
             tc.tile_pool(name="xqp",
                          bufs=(2 if (unroll > 1 or reps > 1) else 1)) as xqp, \
             tc.tile_pool(name="wqp", bufs=wq_bufs) as wqp, \
             tc.tile_pool(name="prtp", bufs=max(ramp_nt, 1)) as prtp, \
             tc.tile_pool(name="outp", bufs=out_bufs) as outp, \
             tc.tile_pool(name="psp",
                          bufs=max(2, min(8, (8 * 512) // max(bg, 512))),
                          space="PSUM") as psp:

            def load_bias():
                bias_bf = const.tile([P, nt], dt.bfloat16)
                nc.sync.dma_start(out=bias_bf[:, :], in_=bvec[:, :])
                bias_sb = const.tile([P, nt], dt.float32)
                nc.vector.tensor_copy(bias_sb[:, :], bias_bf[:, :])
                return bias_sb

            def load_x(xq):
                # stream fp8 straight into the resident tile, xg k-subtiles
                # per DMA (xg*b_sh contiguous bytes per partition). One DMA
                # ring can't keep up with the ramp's k-loop (3.2us/group DMA
                # vs 1.7us/2-k-pairs of MM), so alternate groups across the
                # SP and Activation rings; group 0 is split in half so the
                # first MMs start after only xg/2 subtiles land.
                if not x_two_rings:
                    for g in range(nxg):
                        nc.sync.dma_start(out=xq[:, g * xg:(g + 1) * xg, :],
                                          in_=xt[g])
                    return
                h = xg // 2
                nc.sync.dma_start(out=xq[:, 0:h, :], in_=xt[0][:, 0:h, :])
                nc.scalar.dma_start(out=xq[:, h:xg, :], in_=xt[0][:, h:xg, :])
                for g in range(1, nxg):
                    ring = nc.sync if g % 2 == 0 else nc.scalar
                    ring.dma_start(out=xq[:, g * xg:(g + 1) * xg, :],
                                   in_=xt[g])

            def load_w(n):
                # weight n-tile: fp8 [P, ks, P] (4KB/partition contiguous) on
                # scalar's HWDGE ring so w loads don't queue behind x loads.
                wq = wqp.tile([P, ks, P], dt.float8e4, name="wq")
                nc.scalar.dma_start(out=wq[:, :, :], in_=wp[n])
                return wq

            # xq is re-allocated per reps-iteration from a 2-buf pool so
            # iteration i+1's x stream prefetches under iteration i's MMs
            # (no per-iteration head bubble in the repeat-timing NEFF).
            _xq_hold = [None]

            def new_xq():
                _xq_hold[0] = xqp.tile([P, ks, b_sh], dt.float8e4, name="xq")
                return _xq_hold[0]

            pm = (mybir.MatmulPerfMode.DoubleRow if pmode == "dr"
                  else mybir.MatmulPerfMode.DoubleRowSwInterleave)

            def mm(wq, ps, g, k, start, stop):
                xq = _xq_hold[0]
                nc.tensor.matmul(
                    ps[:, :],
                    lhsT=wq[:, 2 * k:2 * k + 2, :],
                    rhs=xq[:, 2 * k:2 * k + 2, g * bg:(g + 1) * bg],
                    start=start, stop=stop,
                    perf_mode=pm)

            def ldw(wq, k):
                # explicit stationary load; probes whether walrus pairs it
                # with the following matmuls instead of re-loading per MM
                nc.tensor.ldweights(wq[:, 2 * k:2 * k + 2, :], perf_mode=pm)

            hoisted = probe in ("pe", "pe+w", "pe0", "peld")
            bias_sb = load_bias()
            wq0 = None
            if hoisted:
                load_x(new_xq())
                if probe in ("pe", "pe0", "peld"):
                    wq0 = load_w(0)

            with (tc.For_i(0, reps, 1) if reps > 1
                  else contextlib.nullcontext()):
              for _rep in range(unroll):
                if not hoisted and probe == "dma":
                    load_x(new_xq())

                if probe == "dma":
                    # DMAs only: x (above) + w stream + out stores
                    junk = const.tile([P, b_sh], dt.bfloat16, name="junk")
                    nc.vector.memzero(junk[:, :])
                    for n in range(nt):
                        load_w(n)
                        ring = nc.sync if out_ring == "sync" else nc.scalar
                        ring.dma_start(out=out[n], in_=junk[:, :])
                else:
                    # Ramp phase: while x streams in, run first-half-K
                    # accumulation for the first ramp_nt n-tiles (uses only
                    # the first half of x); partials park in SBUF f32.
                    # Tile-serial with k inner; the 8 PSUM banks keep two
                    # tiles in flight so tile n+1's MMs overlap tile n's
                    # park copies. Emission order matters for ring FIFO: w0
                    # first (needed by the first LDW), then the x stream
                    # with the remaining ramp weights interleaved.
                    half = kk // 2
                    ramp_wq, ramp_part = {}, {}
                    ramp_wq[0] = load_w(0)
                    wi = 1
                    if not hoisted:
                        xq = new_xq()
                        if x_two_rings:
                            # interleave the remaining ramp weights between
                            # the odd (Activation-ring) x groups so each w_n
                            # lands just before tile n's k-loop needs it
                            h = xg // 2
                            nc.sync.dma_start(out=xq[:, 0:h, :],
                                              in_=xt[0][:, 0:h, :])
                            nc.scalar.dma_start(out=xq[:, h:xg, :],
                                                in_=xt[0][:, h:xg, :])
                            for g in range(1, nxg):
                                ring = nc.sync if g % 2 == 0 else nc.scalar
                                ring.dma_start(
                                    out=xq[:, g * xg:(g + 1) * xg, :],
                                    in_=xt[g])
                                if g % 2 == 1 and wi < ramp_nt:
                                    ramp_wq[wi] = load_w(wi)
                                    wi += 1
                        else:
                            load_x(xq)
                    while wi < ramp_nt:
                        ramp_wq[wi] = load_w(wi)
                        wi += 1
                    for n in range(ramp_nt):
                        ramp_part[n] = prtp.tile([P, b_sh], dt.float32,
                                                 name="part")
                        pss = [psp.tile([P, bg], mybir.dt.float32, name="ps")
                               for _ in range(nbg)]
                        for k in range(half):
                            for g in range(nbg):
                                mm(ramp_wq[n], pss[g], g, k,
                                   k == 0, k == half - 1)
                        for g in range(nbg):
                            dst = ramp_part[n][:, g * bg:(g + 1) * bg]
                            if epi_split and g % 2:
                                nc.scalar.copy(dst, pss[g][:, :])
                            else:
                                nc.vector.tensor_copy(dst, pss[g][:, :])

                    for n in range(nt):
                        ramp = n < ramp_nt
                        if probe in ("pe", "pe0", "peld"):
                            wq = wq0
                        elif ramp:
                            wq = ramp_wq[n]
                        else:
                            wq = load_w(n)

                        out_sb = outp.tile([P, b_sh], dt.bfloat16,
                                           name="out_sb")
                        k_lo = half if ramp else 0
                        last = n == nt - 1
                        ring = nc.sync if out_ring == "sync" else nc.scalar

                        def epi(g, ps, store):
                            dst = out_sb[:, g * bg:(g + 1) * bg]
                            if ramp:
                                # (psum + bias) + first-half partial -> bf16
                                # (DVE only; Activation has no 3-operand op)
                                nc.vector.scalar_tensor_tensor(
                                    dst, ps[:, :], bias_sb[:, n:n + 1],
                                    ramp_part[n][:, g * bg:(g + 1) * bg],
                                    mybir.AluOpType.add, mybir.AluOpType.add)
                            elif epi_split and g % 2:
                                nc.scalar.add(dst, ps[:, :],
                                              bias_sb[:, n:n + 1])
                            else:
                                nc.vector.tensor_scalar_add(
                                    dst, ps[:, :], bias_sb[:, n:n + 1])
                            if store:
                                ring.dma_start(
                                    out=out[n][:, g * bg:(g + 1) * bg],
                                    in_=dst)

                        if last and tail_split and probe == "":
                            # Final n-tile: g OUTER, k inner, so each batch
                            # group's epilogue + store chunk pipelines under
                            # the next group's MMs; the kernel tail is one
                            # group's epilogue + one 128KB store instead of
                            # the whole tile's.
                            for g in range(nbg):
                                ps = psp.tile([P, bg], mybir.dt.float32,
                                              name="ps")
                                for k in range(k_lo, kk):
                                    mm(wq, ps, g, k, k == k_lo, k == kk - 1)
                                epi(g, ps, True)
                            continue

                        pss = [psp.tile([P, bg], mybir.dt.float32, name="ps")
                               for _ in range(nbg)]
                        for k in range(k_lo, kk):
                            if probe == "peld":
                                ldw(wq, k)
                            for g in range(nbg):
                                mm(wq, pss[g], g, k, k == k_lo, k == kk - 1)
                        if probe == "pe0" and n != nt - 1:
                            continue
                        for g in range(nbg):
                            epi(g, pss[g], False)
                        if probe not in ("pe", "pe+w", "peld") or last:
                            ring.dma_start(out=out[n], in_=out_sb[:, :])

    nc.finalize()
    return nc


def make_key(reps=1):
    b_sh = BATCH // N_CORES
    return (b_sh, K_PAD // P, N_PAD // P, 512, reps)


def _get_nc(key):
    if key not in _kernel_cache:
        nc = _build(*key)
        if DEDUP_LDW:
            _dedup_ldweights(nc)
        _kernel_cache[key] = nc
    return _kernel_cache[key]


def kernel(x, weight, weight_scale, bias):
    global LAST_RESULTS
    from concourse.bass_utils import run_bass_kernel_spmd

    x = np.asarray(x, dtype=np.float32)
    weight = np.asarray(weight, dtype=np.float32)
    weight_scale = np.asarray(weight_scale, dtype=np.float32)
    bias = np.asarray(bias)  # bf16

    n_out, k_pad = weight.shape          # 4032, 4096
    batch, in_f = x.shape                # 16384, 4000
    assert k_pad == K_PAD and batch == BATCH

    b_sh = batch // N_CORES
    ks = K_PAD // P
    nt = N_PAD // P
    xg = 4
    f8 = ml_dtypes.float8_e4m3

    # Quantize weight on host, exactly as the reference does; fold non-one
    # block scales in post-quantization (exact for power-of-two scales).
    wq8 = weight.astype(ml_dtypes.float8_e4m3fn)
    if not np.allclose(weight_scale, 1.0):
        ws = np.repeat(np.repeat(weight_scale, P, axis=0), P, axis=1)
        wq8 = (wq8.astype(np.float32) * ws[:n_out, :k_pad]).astype(
            ml_dtypes.float8_e4m3fn)
    wpad = np.zeros((N_PAD, K_PAD), dtype=f8)
    wpad[:n_out] = wq8.view(np.uint8).view(f8)
    # w -> [nt, p, ks, j]: element = w[nt*128 + j, ks*128 + p], zero-pad rows
    wp = np.ascontiguousarray(wpad.reshape(nt, P, ks, P).transpose(0, 3, 2, 1))

    # x: quantize once, pad features to K_PAD
    xq8 = np.zeros((batch, K_PAD), dtype=f8)
    xq8[:, :in_f] = x.astype(ml_dtypes.float8_e4m3fn).view(np.uint8).view(f8)

    # bias -> [p, nt] bf16, zero-padded
    bpad = np.zeros(N_PAD, dtype=ml_dtypes.bfloat16)
    bpad[:n_out] = bias
    bvec = np.ascontiguousarray(bpad.reshape(nt, P).T)

    in_maps = []
    for c in range(N_CORES):
        shard = xq8[c * b_sh:(c + 1) * b_sh]        # [b_sh, K_PAD] fp8
        # -> [nxg, P, xg, b_sh]: element (g,p,j,b) = x[b, (g*xg+j)*P + p]
        xt = np.ascontiguousarray(
            shard.T.reshape(ks // xg, xg, P, b_sh).transpose(0, 2, 1, 3))
        in_maps.append({"xt": xt, "wp": wp, "bvec": bvec})

    global _last_in_maps
    _last_in_maps = in_maps
    nc = _get_nc(make_key(1))
    res = run_bass_kernel_spmd(nc, in_maps, list(range(N_CORES)), trace=TRACE)
    LAST_RESULTS = res

    final = np.empty((batch, OUT_F), dtype=ml_dtypes.bfloat16)
    for c in range(N_CORES):
        oc = res.results[c]["out"].reshape(N_PAD, b_sh)
        final[c * b_sh:(c + 1) * b_sh, :] = oc[:OUT_F].T
    return final



# revision 28
# speedup vs baseline: 1.3939x; 1.0016x over previous
"""DeepGEMM-style fp8 linear on 8 TRN2 NeuronCores.

Computes: out = bf16( fp8(x_pad) @ (fp8(W) * block_scale).T ) + bias, sliced to
[16384, 4000], matching the jax reference (block scales are ones, bias zeros).

Strategy: batch-parallel SPMD with HOST-side fp8 quantization. Each core gets
a 2048-row batch shard of x, pre-quantized to fp8_e4m3 and transposed to
[k, b] on host, plus the full weight pre-quantized and transposed to [k, n]
blocks. fp8 quantization on host is bit-identical to the reference's
float8_e4m3fn round-trip for this value range (verified: e4m3 and e4m3fn
encodings coincide below the e4m3 max). On device: stream fp8 tiles, fp8
matmul with DoubleRow perf mode accumulating in fp32 PSUM, add bias + cast
to bf16, store out as [n, b]; host transposes/concats the shards back.

Why: with f32 inputs the kernel moves 117MB/core (DMA-floor ~330-350us);
with fp8 inputs it moves 42MB/core (~120us), making the kernel PE-bound.
HW-measured fp8 DoubleRow runs ~190ns per [128,512] matmul (0.85 cyc/col
incl. the per-matmul LDWEIGHTS, which walrus emits per instruction with
ldw-opt disabled) => ~390us of PE for the 2048-matmul schedule; the x-load
head bubble (~24us, DMA-paced) is absorbed by a ramp phase that computes
first-half-K partials for the first ramp_nt n-tiles while x streams in.
Measured 361-483us per exec depending on device contention (baseline 628us).
PSUM matmul output is ISA-capped at 512 f32 (one bank) => bg=512, 4 groups,
8-bank rotation. DoubleRowSwInterleave mis-computes with this weight layout
and is slower in-situ; plain DoubleRow is the right mode.
"""

import sys

if "/opt/trn_rl_repo" not in sys.path:
    sys.path.insert(0, "/opt/trn_rl_repo")

import numpy as np
import ml_dtypes

P = 128
N_CORES = 8
BATCH = 16384
IN_F = 4000
OUT_F = 4000
K_PAD = 4096               # in-features padded to 32 k-subtiles of 128
N_PAD = 4096               # out-features padded 4032 -> 4096 (uniform n-tiles)

_kernel_cache = {}

# test.py knobs
TRACE = False
LAST_RESULTS = None

# Bass lowers every matmul into a standalone InstLdweights + InstMatmult
# (ldweights=False) pair; a DoubleRow LDWEIGHTS streams 256 weight columns
# (~200ns), which matches or exceeds the ~107-213ns MM streaming time, so
# per-MM reloads of the SAME weight pair (our g-loop reuses each weight 4x)
# make the weight-load path the PE bottleneck. walrus --enable-ldw-opt
# rejects DoubleRow, so dedup at the BIR level instead: drop an
# InstLdweights when it is sync-free and reloads exactly what the previous
# (kept) LDW already put in the array. MMs never clobber array weights;
# any other PE-engine instruction resets the tracked state.
DEDUP_LDW = True


def _dedup_ldweights(nc):
    from concourse import mybir
    removed = kept = 0
    for b in nc.m.functions[0].blocks:
        keep = []
        last_key = None
        for i in b.instructions:
            if isinstance(i, mybir.InstLdweights):
                si = i.sync_info
                clean = si is None or (not si.on_wait and not si.on_update)
                key = (str(i.ins[0]), str(i.perf_mode), str(i.tile_position),
                       str(i.tile_size), i.is_transpose)
                if clean and key == last_key:
                    removed += 1
                    continue
                last_key = key
                kept += 1
            elif isinstance(i, mybir.InstMatmult):
                assert i.ldweights is False
                if i.is_transpose:        # transpose-mode clobbers the array
                    last_key = None
            elif getattr(i, "engine", None) == mybir.EngineType.PE:
                last_key = None
            keep.append(i)
        if removed:
            del b.instructions[:]
            b.instructions.extend(keep)
    return removed, kept


def _build(b_sh, ks, nt, bg, reps=1, xg=4, ramp_nt=0, wq_bufs=6, out_bufs=2,
           out_ring="sync", epi_split=True, probe="", unroll=1, pmode="dr",
           x_two_rings=True, tail_split=True):
    """probe: '' normal kernel; 'pe' = x+w0 hoisted out of the reps loop,
    matmuls+epilogue only (pure PE rate); 'pe+w' = x hoisted, w streamed;
    'dma' = DMAs only, no compute (pure DMA rate).
    unroll: python-level body repetition (for TimelineSim, which cannot
    resolve For_i branches)."""
    import contextlib
    from concourse import bacc, tile, mybir
    from concourse.mybir import dt

    nbg = b_sh // bg
    assert nbg * bg == b_sh
    nxg = ks // xg
    assert nxg * xg == ks
    kk = ks // 2                      # DoubleRow k-pairs
    nc = bacc.Bacc(None, target_bir_lowering=False, debug=False)

    with tile.TileContext(nc) as tc:
        with tc.tile_pool(name="dram", bufs=1, space="DRAM") as dram:
            xt = dram.tile([nxg, P, xg, b_sh], dt.float8e4, kind="ExternalInput",
                           name="xt", uniquify=False)
            wp = dram.tile([nt, P, ks, P], dt.float8e4, kind="ExternalInput",
                           name="wp", uniquify=False)
            bvec = dram.tile([P, nt], dt.bfloat16, kind="ExternalInput",
                             name="bvec", uniquify=False)
            out = dram.tile([nt, P, b_sh], dt.bfloat16, kind="ExternalOutput",
                            name="out", uniquify=False)

        with tc.tile_pool(name="const", bufs=1) as const, \
             tc.tile_pool(name="xqp",
                          bufs=(2 if (unroll > 1 or reps > 1) else 1)) as xqp, \
             tc.tile_pool(name="wqp", bufs=wq_bufs) as wqp, \
             tc.tile_pool(name="prtp", bufs=max(ramp_nt, 1)) as prtp, \
             tc.tile_pool(name="outp", bufs=out_bufs) as outp, \
             tc.tile_pool(name="psp",
                          bufs=max(2, min(8, (8 * 512) // max(bg, 512))),
                          space="PSUM") as psp:

            def load_bias():
                bias_bf = const.tile([P, nt], dt.bfloat16)
                nc.sync.dma_start(out=bias_bf[:, :], in_=bvec[:, :])
                bias_sb = const.tile([P, nt], dt.float32)
                nc.vector.tensor_copy(bias_sb[:, :], bias_bf[:, :])
                return bias_sb

            def load_x(xq):
                # stream fp8 straight into the resident tile, xg k-subtiles
                # per DMA (xg*b_sh contiguous bytes per partition). One DMA
                # ring can't keep up with the ramp's k-loop (3.2us/group DMA
                # vs 1.7us/2-k-pairs of MM), so alternate groups across the
                # SP and Activation rings; group 0 is split in half so the
                # first MMs start after only xg/2 subtiles land.
                if not x_two_rings:
                    for g in range(nxg):
                        nc.sync.dma_start(out=xq[:, g * xg:(g + 1) * xg, :],
                                          in_=xt[g])
                    return
                h = xg // 2
                nc.sync.dma_start(out=xq[:, 0:h, :], in_=xt[0][:, 0:h, :])
                nc.scalar.dma_start(out=xq[:, h:xg, :], in_=xt[0][:, h:xg, :])
                for g in range(1, nxg):
                    ring = nc.sync if g % 2 == 0 else nc.scalar
                    ring.dma_start(out=xq[:, g * xg:(g + 1) * xg, :],
                                   in_=xt[g])

            def load_w(n):
                # weight n-tile: fp8 [P, ks, P] (4KB/partition contiguous) on
                # scalar's HWDGE ring so w loads don't queue behind x loads.
                wq = wqp.tile([P, ks, P], dt.float8e4, name="wq")
                nc.scalar.dma_start(out=wq[:, :, :], in_=wp[n])
                return wq

            # xq is re-allocated per reps-iteration from a 2-buf pool so
            # iteration i+1's x stream prefetches under iteration i's MMs
            # (no per-iteration head bubble in the repeat-timing NEFF).
            _xq_hold = [None]

            def new_xq():
                _xq_hold[0] = xqp.tile([P, ks, b_sh], dt.float8e4, name="xq")
                return _xq_hold[0]

            pm = (mybir.MatmulPerfMode.DoubleRow if pmode == "dr"
                  else mybir.MatmulPerfMode.DoubleRowSwInterleave)

            def mm(wq, ps, g, k, start, stop):
                xq = _xq_hold[0]
                nc.tensor.matmul(
                    ps[:, :],
                    lhsT=wq[:, 2 * k:2 * k + 2, :],
                    rhs=xq[:, 2 * k:2 * k + 2, g * bg:(g + 1) * bg],
                    start=start, stop=stop,
                    perf_mode=pm)

            def ldw(wq, k):
                # explicit stationary load; probes whether walrus pairs it
                # with the following matmuls instead of re-loading per MM
                nc.tensor.ldweights(wq[:, 2 * k:2 * k + 2, :], perf_mode=pm)

            hoisted = probe in ("pe", "pe+w", "pe0", "peld")
            bias_sb = load_bias()
            wq0 = None
            if hoisted:
                load_x(new_xq())
                if probe in ("pe", "pe0", "peld"):
                    wq0 = load_w(0)

            with (tc.For_i(0, reps, 1) if reps > 1
                  else contextlib.nullcontext()):
              for _rep in range(unroll):
                if not hoisted and probe == "dma":
                    load_x(new_xq())

                if probe == "dma":
                    # DMAs only: x (above) + w stream + out stores
                    junk = const.tile([P, b_sh], dt.bfloat16, name="junk")
                    nc.vector.memzero(junk[:, :])
                    for n in range(nt):
                        load_w(n)
                        ring = nc.sync if out_ring == "sync" else nc.scalar
                        ring.dma_start(out=out[n], in_=junk[:, :])
                else:
                    # Ramp phase: while x streams in, run first-half-K
                    # accumulation for the first ramp_nt n-tiles (uses only
                    # the first half of x); partials park in SBUF f32.
                    # Tile-serial with k inner; the 8 PSUM banks keep two
                    # tiles in flight so tile n+1's MMs overlap tile n's
                    # park copies. Emission order matters for ring FIFO: w0
                    # first (needed by the first LDW), then the x stream
                    # with the remaining ramp weights interleaved.
                    half = kk // 2
                    ramp_wq, ramp_part = {}, {}
                    if ramp_nt > 0:
                        ramp_wq[0] = load_w(0)
                    wi = 1
                    if not hoisted:
                        xq = new_xq()
                        if x_two_rings:
                            # interleave the remaining ramp weights between
                            # the odd (Activation-ring) x groups so each w_n
                            # lands just before tile n's k-loop needs it
                            h = xg // 2
                            nc.sync.dma_start(out=xq[:, 0:h, :],
                                              in_=xt[0][:, 0:h, :])
                            nc.scalar.dma_start(out=xq[:, h:xg, :],
                                                in_=xt[0][:, h:xg, :])
                            for g in range(1, nxg):
                                ring = nc.sync if g % 2 == 0 else nc.scalar
                                ring.dma_start(
                                    out=xq[:, g * xg:(g + 1) * xg, :],
                                    in_=xt[g])
                                if g % 2 == 1 and wi < ramp_nt:
                                    ramp_wq[wi] = load_w(wi)
                                    wi += 1
                        else:
                            load_x(xq)
                    while wi < ramp_nt:
                        ramp_wq[wi] = load_w(wi)
                        wi += 1
                    for n in range(ramp_nt):
                        ramp_part[n] = prtp.tile([P, b_sh], dt.float32,
                                                 name="part")
                        pss = [psp.tile([P, bg], mybir.dt.float32, name="ps")
                               for _ in range(nbg)]
                        for k in range(half):
                            for g in range(nbg):
                                mm(ramp_wq[n], pss[g], g, k,
                                   k == 0, k == half - 1)
                        for g in range(nbg):
                            dst = ramp_part[n][:, g * bg:(g + 1) * bg]
                            if epi_split and g % 2:
                                nc.scalar.copy(dst, pss[g][:, :])
                            else:
                                nc.vector.tensor_copy(dst, pss[g][:, :])

                    for n in range(nt):
                        ramp = n < ramp_nt
                        if probe in ("pe", "pe0", "peld"):
                            wq = wq0
                        elif ramp:
                            wq = ramp_wq[n]
                        else:
                            wq = load_w(n)

                        out_sb = outp.tile([P, b_sh], dt.bfloat16,
                                           name="out_sb")
                        k_lo = half if ramp else 0
                        last = n == nt - 1
                        ring = nc.sync if out_ring == "sync" else nc.scalar

                        def epi(g, ps, store):
                            dst = out_sb[:, g * bg:(g + 1) * bg]
                            if ramp:
                                # (psum + bias) + first-half partial -> bf16
                                # (DVE only; Activation has no 3-operand op)
                                nc.vector.scalar_tensor_tensor(
                                    dst, ps[:, :], bias_sb[:, n:n + 1],
                                    ramp_part[n][:, g * bg:(g + 1) * bg],
                                    mybir.AluOpType.add, mybir.AluOpType.add)
                            elif epi_split and g % 2:
                                nc.scalar.add(dst, ps[:, :],
                                              bias_sb[:, n:n + 1])
                            else:
                                nc.vector.tensor_scalar_add(
                                    dst, ps[:, :], bias_sb[:, n:n + 1])
                            if store:
                                ring.dma_start(
                                    out=out[n][:, g * bg:(g + 1) * bg],
                                    in_=dst)

                        if last and tail_split and probe == "":
                            # Final n-tile: g OUTER, k inner, so each batch
                            # group's epilogue + store chunk pipelines under
                            # the next group's MMs; the kernel tail is one
                            # group's epilogue + one 128KB store instead of
                            # the whole tile's.
                            for g in range(nbg):
                                ps = psp.tile([P, bg], mybir.dt.float32,
                                              name="ps")
                                for k in range(k_lo, kk):
                                    mm(wq, ps, g, k, k == k_lo, k == kk - 1)
                                epi(g, ps, True)
                            continue

                        pss = [psp.tile([P, bg], mybir.dt.float32, name="ps")
                               for _ in range(nbg)]
                        for k in range(k_lo, kk):
                            if probe == "peld":
                                ldw(wq, k)
                            for g in range(nbg):
                                mm(wq, pss[g], g, k, k == k_lo, k == kk - 1)
                        if probe == "pe0" and n != nt - 1:
                            continue
                        for g in range(nbg):
                            epi(g, pss[g], False)
                        if probe not in ("pe", "pe+w", "peld") or last:
                            ring.dma_start(out=out[n], in_=out_sb[:, :])

    nc.finalize()
    return nc


def make_key(reps=1):
    b_sh = BATCH // N_CORES
    return (b_sh, K_PAD // P, N_PAD // P, 512, reps)


def _get_nc(key):
    if key not in _kernel_cache:
        nc = _build(*key)
        if DEDUP_LDW:
            _dedup_ldweights(nc)
        _kernel_cache[key] = nc
    return _kernel_cache[key]


def kernel(x, weight, weight_scale, bias):
    global LAST_RESULTS
    from concourse.bass_utils import run_bass_kernel_spmd

    x = np.asarray(x, dtype=np.float32)
    weight = np.asarray(weight, dtype=np.float32)
    weight_scale = np.asarray(weight_scale, dtype=np.float32)
    bias = np.asarray(bias)  # bf16

    n_out, k_pad = weight.shape          # 4032, 4096
    batch, in_f = x.shape                # 16384, 4000
    assert k_pad == K_PAD and batch == BATCH

    b_sh = batch // N_CORES
    ks = K_PAD // P
    nt = N_PAD // P
    xg = 4
    f8 = ml_dtypes.float8_e4m3

    # Quantize weight on host, exactly as the reference does; fold non-one
    # block scales in post-quantization (exact for power-of-two scales).
    wq8 = weight.astype(ml_dtypes.float8_e4m3fn)
    if not np.allclose(weight_scale, 1.0):
        ws = np.repeat(np.repeat(weight_scale, P, axis=0), P, axis=1)
        wq8 = (wq8.astype(np.float32) * ws[:n_out, :k_pad]).astype(
            ml_dtypes.float8_e4m3fn)
    wpad = np.zeros((N_PAD, K_PAD), dtype=f8)
    wpad[:n_out] = wq8.view(np.uint8).view(f8)
    # w -> [nt, p, ks, j]: element = w[nt*128 + j, ks*128 + p], zero-pad rows
    wp = np.ascontiguousarray(wpad.reshape(nt, P, ks, P).transpose(0, 3, 2, 1))

    # x: quantize once, pad features to K_PAD
    xq8 = np.zeros((batch, K_PAD), dtype=f8)
    xq8[:, :in_f] = x.astype(ml_dtypes.float8_e4m3fn).view(np.uint8).view(f8)

    # bias -> [p, nt] bf16, zero-padded
    bpad = np.zeros(N_PAD, dtype=ml_dtypes.bfloat16)
    bpad[:n_out] = bias
    bvec = np.ascontiguousarray(bpad.reshape(nt, P).T)

    in_maps = []
    for c in range(N_CORES):
        shard = xq8[c * b_sh:(c + 1) * b_sh]        # [b_sh, K_PAD] fp8
        # -> [nxg, P, xg, b_sh]: element (g,p,j,b) = x[b, (g*xg+j)*P + p]
        xt = np.ascontiguousarray(
            shard.T.reshape(ks // xg, xg, P, b_sh).transpose(0, 2, 1, 3))
        in_maps.append({"xt": xt, "wp": wp, "bvec": bvec})

    global _last_in_maps
    _last_in_maps = in_maps
    nc = _get_nc(make_key(1))
    res = run_bass_kernel_spmd(nc, in_maps, list(range(N_CORES)), trace=TRACE)
    LAST_RESULTS = res

    final = np.empty((batch, OUT_F), dtype=ml_dtypes.bfloat16)
    for c in range(N_CORES):
        oc = res.results[c]["out"].reshape(N_PAD, b_sh)
        final[c * b_sh:(c + 1) * b_sh, :] = oc[:OUT_F].T
    return final



# revision 31
# speedup vs baseline: 1.5966x; 1.1455x over previous
"""DeepGEMM-style fp8 linear on 8 TRN2 NeuronCores.

Computes: out = bf16( fp8(x_pad) @ (fp8(W) * block_scale).T ) + bias, sliced to
[16384, 4000], matching the jax reference (block scales are ones, bias zeros).

Strategy: batch-parallel SPMD with HOST-side fp8 quantization. Each core gets
a 2048-row batch shard of x, pre-quantized to fp8_e4m3 and transposed to
[k, b] on host, plus the full weight pre-quantized and transposed to [k, n]
blocks. fp8 quantization on host is bit-identical to the reference's
float8_e4m3fn round-trip for this value range (verified: e4m3 and e4m3fn
encodings coincide below the e4m3 max). On device: stream fp8 tiles, fp8
matmul with DoubleRow perf mode accumulating in fp32 PSUM, add bias + cast
to bf16, store out as [n, b]; host transposes/concats the shards back.

Schedule (empirically tuned on HW; each item A/B-measured):
- bg=256: 8 batch groups per (n-tile, k-pair) => each DoubleRow weight
  pair serves 8 consecutive matmuls, amortizing the LDWEIGHTS stream.
- BIR-level LDWEIGHTS dedup (_dedup_ldweights): bass emits a standalone
  InstLdweights per matmul; sync-free reloads of the identical weight AP
  are dropped (~73%), worth ~75us/exec (walrus --enable-ldw-opt rejects
  DoubleRow, so the dedup is done here instead).
- epilogue split across DVE and Activation engines (PSUM->SBUF bias+cast);
  single-engine epilogue costs ~90us/exec in PSUM/port contention.
- x double-buffered ACROSS For_i iterations (2-buf pool): iteration i+1's
  8MB x stream prefetches under iteration i's matmuls, so the steady-state
  body has no x-load head; x groups alternate the SP and Activation DGE
  rings. No ramp phase (the old ramp's park copies cost DVE/PSUM traffic
  every iteration to hide a head that only exists in iteration 0).
- out stores on the SP ring overlap under the matmul shadow.
Measured (repeat-differenced, 9-exec bursts to stay under the chip's
power-throttle onset): ~266-365us/exec depending on device contention
(session baseline 433746ns; PE fp8-DoubleRow naive roofline would be
437us at 2.4GHz -- the array sustains ~0.6-0.7 cyc/column-pair in this
schedule). DoubleRowSwInterleave mis-computes with this weight layout and
is slower in-situ; plain DoubleRow is the right mode.
"""

import sys

if "/opt/trn_rl_repo" not in sys.path:
    sys.path.insert(0, "/opt/trn_rl_repo")

import numpy as np
import ml_dtypes

P = 128
N_CORES = 8
BATCH = 16384
IN_F = 4000
OUT_F = 4000
K_PAD = 4096               # in-features padded to 32 k-subtiles of 128
N_PAD = 4096               # out-features padded 4032 -> 4096 (uniform n-tiles)

_kernel_cache = {}

# test.py knobs
TRACE = False
LAST_RESULTS = None

# Bass lowers every matmul into a standalone InstLdweights + InstMatmult
# (ldweights=False) pair; a DoubleRow LDWEIGHTS streams 256 weight columns
# (~200ns), which matches or exceeds the ~107-213ns MM streaming time, so
# per-MM reloads of the SAME weight pair (our g-loop reuses each weight 4x)
# make the weight-load path the PE bottleneck. walrus --enable-ldw-opt
# rejects DoubleRow, so dedup at the BIR level instead: drop an
# InstLdweights when it is sync-free and reloads exactly what the previous
# (kept) LDW already put in the array. MMs never clobber array weights;
# any other PE-engine instruction resets the tracked state.
DEDUP_LDW = True


def _dedup_ldweights(nc):
    from concourse import mybir
    removed = kept = 0
    for b in nc.m.functions[0].blocks:
        keep = []
        last_key = None
        for i in b.instructions:
            if isinstance(i, mybir.InstLdweights):
                si = i.sync_info
                clean = si is None or (not si.on_wait and not si.on_update)
                key = (str(i.ins[0]), str(i.perf_mode), str(i.tile_position),
                       str(i.tile_size), i.is_transpose)
                if clean and key == last_key:
                    removed += 1
                    continue
                last_key = key
                kept += 1
            elif isinstance(i, mybir.InstMatmult):
                assert i.ldweights is False
                if i.is_transpose:        # transpose-mode clobbers the array
                    last_key = None
            elif getattr(i, "engine", None) == mybir.EngineType.PE:
                last_key = None
            keep.append(i)
        if removed:
            del b.instructions[:]
            b.instructions.extend(keep)
    return removed, kept


def _build(b_sh, ks, nt, bg, reps=1, xg=4, ramp_nt=0, wq_bufs=8, out_bufs=2,
           out_ring="sync", epi_split=True, probe="", unroll=1, pmode="dr",
           x_two_rings=True, tail_split=False):
    """probe: '' normal kernel; 'pe' = x+w0 hoisted out of the reps loop,
    matmuls+epilogue only (pure PE rate); 'pe+w' = x hoisted, w streamed;
    'dma' = DMAs only, no compute (pure DMA rate).
    unroll: python-level body repetition (for TimelineSim, which cannot
    resolve For_i branches)."""
    import contextlib
    from concourse import bacc, tile, mybir
    from concourse.mybir import dt

    nbg = b_sh // bg
    assert nbg * bg == b_sh
    nxg = ks // xg
    assert nxg * xg == ks
    kk = ks // 2                      # DoubleRow k-pairs
    nc = bacc.Bacc(None, target_bir_lowering=False, debug=False)

    with tile.TileContext(nc) as tc:
        with tc.tile_pool(name="dram", bufs=1, space="DRAM") as dram:
            xt = dram.tile([nxg, P, xg, b_sh], dt.float8e4, kind="ExternalInput",
                           name="xt", uniquify=False)
            wp = dram.tile([nt, P, ks, P], dt.float8e4, kind="ExternalInput",
                           name="wp", uniquify=False)
            bvec = dram.tile([P, nt], dt.bfloat16, kind="ExternalInput",
                             name="bvec", uniquify=False)
            out = dram.tile([nt, P, b_sh], dt.bfloat16, kind="ExternalOutput",
                            name="out", uniquify=False)

        with tc.tile_pool(name="const", bufs=1) as const, \
             tc.tile_pool(name="xqp",
                          bufs=(2 if (unroll > 1 or reps > 1) else 1)) as xqp, \
             tc.tile_pool(name="wqp", bufs=wq_bufs) as wqp, \
             tc.tile_pool(name="prtp", bufs=max(ramp_nt, 1)) as prtp, \
             tc.tile_pool(name="outp", bufs=out_bufs) as outp, \
             tc.tile_pool(name="psp",
                          bufs=max(2, min(8, (8 * 512) // max(bg, 512))),
                          space="PSUM") as psp:

            def load_bias():
                bias_bf = const.tile([P, nt], dt.bfloat16)
                nc.sync.dma_start(out=bias_bf[:, :], in_=bvec[:, :])
                bias_sb = const.tile([P, nt], dt.float32)
                nc.vector.tensor_copy(bias_sb[:, :], bias_bf[:, :])
                return bias_sb

            def load_x(xq):
                # stream fp8 straight into the resident tile, xg k-subtiles
                # per DMA (xg*b_sh contiguous bytes per partition). One DMA
                # ring can't keep up with the ramp's k-loop (3.2us/group DMA
                # vs 1.7us/2-k-pairs of MM), so alternate groups across the
                # SP and Activation rings; group 0 is split in half so the
                # first MMs start after only xg/2 subtiles land.
                if not x_two_rings:
                    for g in range(nxg):
                        nc.sync.dma_start(out=xq[:, g * xg:(g + 1) * xg, :],
                                          in_=xt[g])
                    return
                h = xg // 2
                nc.sync.dma_start(out=xq[:, 0:h, :], in_=xt[0][:, 0:h, :])
                nc.scalar.dma_start(out=xq[:, h:xg, :], in_=xt[0][:, h:xg, :])
                for g in range(1, nxg):
                    ring = nc.sync if g % 2 == 0 else nc.scalar
                    ring.dma_start(out=xq[:, g * xg:(g + 1) * xg, :],
                                   in_=xt[g])

            def load_w(n):
                # weight n-tile: fp8 [P, ks, P] (4KB/partition contiguous) on
                # scalar's HWDGE ring so w loads don't queue behind x loads.
                wq = wqp.tile([P, ks, P], dt.float8e4, name="wq")
                nc.scalar.dma_start(out=wq[:, :, :], in_=wp[n])
                return wq

            # xq is re-allocated per reps-iteration from a 2-buf pool so
            # iteration i+1's x stream prefetches under iteration i's MMs
            # (no per-iteration head bubble in the repeat-timing NEFF).
            _xq_hold = [None]

            def new_xq():
                _xq_hold[0] = xqp.tile([P, ks, b_sh], dt.float8e4, name="xq")
                return _xq_hold[0]

            pm = (mybir.MatmulPerfMode.DoubleRow if pmode == "dr"
                  else mybir.MatmulPerfMode.DoubleRowSwInterleave)

            def mm(wq, ps, g, k, start, stop):
                xq = _xq_hold[0]
                nc.tensor.matmul(
                    ps[:, :],
                    lhsT=wq[:, 2 * k:2 * k + 2, :],
                    rhs=xq[:, 2 * k:2 * k + 2, g * bg:(g + 1) * bg],
                    start=start, stop=stop,
                    perf_mode=pm)

            def ldw(wq, k):
                # explicit stationary load; probes whether walrus pairs it
                # with the following matmuls instead of re-loading per MM
                nc.tensor.ldweights(wq[:, 2 * k:2 * k + 2, :], perf_mode=pm)

            hoisted = probe in ("pe", "pe+w", "pe0", "peld")
            bias_sb = load_bias()
            wq0 = None
            if hoisted:
                load_x(new_xq())
                if probe in ("pe", "pe0", "peld"):
                    wq0 = load_w(0)

            with (tc.For_i(0, reps, 1) if reps > 1
                  else contextlib.nullcontext()):
              for _rep in range(unroll):
                if not hoisted and probe == "dma":
                    load_x(new_xq())

                if probe == "dma":
                    # DMAs only: x (above) + w stream + out stores
                    junk = const.tile([P, b_sh], dt.bfloat16, name="junk")
                    nc.vector.memzero(junk[:, :])
                    for n in range(nt):
                        load_w(n)
                        ring = nc.sync if out_ring == "sync" else nc.scalar
                        ring.dma_start(out=out[n], in_=junk[:, :])
                else:
                    # Ramp phase: while x streams in, run first-half-K
                    # accumulation for the first ramp_nt n-tiles (uses only
                    # the first half of x); partials park in SBUF f32.
                    # Tile-serial with k inner; the 8 PSUM banks keep two
                    # tiles in flight so tile n+1's MMs overlap tile n's
                    # park copies. Emission order matters for ring FIFO: w0
                    # first (needed by the first LDW), then the x stream
                    # with the remaining ramp weights interleaved.
                    half = kk // 2
                    ramp_wq, ramp_part = {}, {}
                    if ramp_nt > 0:
                        ramp_wq[0] = load_w(0)
                    wi = 1
                    if not hoisted:
                        xq = new_xq()
                        if x_two_rings:
                            # interleave the remaining ramp weights between
                            # the odd (Activation-ring) x groups so each w_n
                            # lands just before tile n's k-loop needs it
                            h = xg // 2
                            nc.sync.dma_start(out=xq[:, 0:h, :],
                                              in_=xt[0][:, 0:h, :])
                            nc.scalar.dma_start(out=xq[:, h:xg, :],
                                                in_=xt[0][:, h:xg, :])
                            for g in range(1, nxg):
                                ring = nc.sync if g % 2 == 0 else nc.scalar
                                ring.dma_start(
                                    out=xq[:, g * xg:(g + 1) * xg, :],
                                    in_=xt[g])
                                if g % 2 == 1 and wi < ramp_nt:
                                    ramp_wq[wi] = load_w(wi)
                                    wi += 1
                        else:
                            load_x(xq)
                    while wi < ramp_nt:
                        ramp_wq[wi] = load_w(wi)
                        wi += 1
                    for n in range(ramp_nt):
                        ramp_part[n] = prtp.tile([P, b_sh], dt.float32,
                                                 name="part")
                        pss = [psp.tile([P, bg], mybir.dt.float32, name="ps")
                               for _ in range(nbg)]
                        for k in range(half):
                            for g in range(nbg):
                                mm(ramp_wq[n], pss[g], g, k,
                                   k == 0, k == half - 1)
                        for g in range(nbg):
                            dst = ramp_part[n][:, g * bg:(g + 1) * bg]
                            if epi_split and g % 2:
                                nc.scalar.copy(dst, pss[g][:, :])
                            else:
                                nc.vector.tensor_copy(dst, pss[g][:, :])

                    for n in range(nt):
                        ramp = n < ramp_nt
                        if probe in ("pe", "pe0", "peld"):
                            wq = wq0
                        elif ramp:
                            wq = ramp_wq[n]
                        else:
                            wq = load_w(n)

                        out_sb = outp.tile([P, b_sh], dt.bfloat16,
                                           name="out_sb")
                        k_lo = half if ramp else 0
                        last = n == nt - 1
                        ring = nc.sync if out_ring == "sync" else nc.scalar

                        def epi(g, ps, store):
                            dst = out_sb[:, g * bg:(g + 1) * bg]
                            if ramp:
                                # (psum + bias) + first-half partial -> bf16
                                # (DVE only; Activation has no 3-operand op)
                                nc.vector.scalar_tensor_tensor(
                                    dst, ps[:, :], bias_sb[:, n:n + 1],
                                    ramp_part[n][:, g * bg:(g + 1) * bg],
                                    mybir.AluOpType.add, mybir.AluOpType.add)
                            elif epi_split and g % 2:
                                nc.scalar.add(dst, ps[:, :],
                                              bias_sb[:, n:n + 1])
                            else:
                                nc.vector.tensor_scalar_add(
                                    dst, ps[:, :], bias_sb[:, n:n + 1])
                            if store:
                                ring.dma_start(
                                    out=out[n][:, g * bg:(g + 1) * bg],
                                    in_=dst)

                        if last and tail_split and probe == "":
                            # Final n-tile: g OUTER, k inner, so each batch
                            # group's epilogue + store chunk pipelines under
                            # the next group's MMs; the kernel tail is one
                            # group's epilogue + one 128KB store instead of
                            # the whole tile's.
                            for g in range(nbg):
                                ps = psp.tile([P, bg], mybir.dt.float32,
                                              name="ps")
                                for k in range(k_lo, kk):
                                    mm(wq, ps, g, k, k == k_lo, k == kk - 1)
                                epi(g, ps, True)
                            continue

                        pss = [psp.tile([P, bg], mybir.dt.float32, name="ps")
                               for _ in range(nbg)]
                        for k in range(k_lo, kk):
                            if probe == "peld":
                                ldw(wq, k)
                            for g in range(nbg):
                                mm(wq, pss[g], g, k, k == k_lo, k == kk - 1)
                        if probe == "pe0" and n != nt - 1:
                            continue
                        for g in range(nbg):
                            epi(g, pss[g], False)
                        if probe not in ("pe", "pe+w", "peld") or last:
                            ring.dma_start(out=out[n], in_=out_sb[:, :])

    nc.finalize()
    return nc


def make_key(reps=1):
    b_sh = BATCH // N_CORES
    return (b_sh, K_PAD // P, N_PAD // P, 256, reps)


def _get_nc(key):
    if key not in _kernel_cache:
        nc = _build(*key)
        if DEDUP_LDW:
            _dedup_ldweights(nc)
        _kernel_cache[key] = nc
    return _kernel_cache[key]


def kernel(x, weight, weight_scale, bias):
    global LAST_RESULTS
    from concourse.bass_utils import run_bass_kernel_spmd

    x = np.asarray(x, dtype=np.float32)
    weight = np.asarray(weight, dtype=np.float32)
    weight_scale = np.asarray(weight_scale, dtype=np.float32)
    bias = np.asarray(bias)  # bf16

    n_out, k_pad = weight.shape          # 4032, 4096
    batch, in_f = x.shape                # 16384, 4000
    assert k_pad == K_PAD and batch == BATCH

    b_sh = batch // N_CORES
    ks = K_PAD // P
    nt = N_PAD // P
    xg = 4
    f8 = ml_dtypes.float8_e4m3

    # Quantize weight on host, exactly as the reference does; fold non-one
    # block scales in post-quantization (exact for power-of-two scales).
    wq8 = weight.astype(ml_dtypes.float8_e4m3fn)
    if not np.allclose(weight_scale, 1.0):
        ws = np.repeat(np.repeat(weight_scale, P, axis=0), P, axis=1)
        wq8 = (wq8.astype(np.float32) * ws[:n_out, :k_pad]).astype(
            ml_dtypes.float8_e4m3fn)
    wpad = np.zeros((N_PAD, K_PAD), dtype=f8)
    wpad[:n_out] = wq8.view(np.uint8).view(f8)
    # w -> [nt, p, ks, j]: element = w[nt*128 + j, ks*128 + p], zero-pad rows
    wp = np.ascontiguousarray(wpad.reshape(nt, P, ks, P).transpose(0, 3, 2, 1))

    # x: quantize once, pad features to K_PAD
    xq8 = np.zeros((batch, K_PAD), dtype=f8)
    xq8[:, :in_f] = x.astype(ml_dtypes.float8_e4m3fn).view(np.uint8).view(f8)

    # bias -> [p, nt] bf16, zero-padded
    bpad = np.zeros(N_PAD, dtype=ml_dtypes.bfloat16)
    bpad[:n_out] = bias
    bvec = np.ascontiguousarray(bpad.reshape(nt, P).T)

    in_maps = []
    for c in range(N_CORES):
        shard = xq8[c * b_sh:(c + 1) * b_sh]        # [b_sh, K_PAD] fp8
        # -> [nxg, P, xg, b_sh]: element (g,p,j,b) = x[b, (g*xg+j)*P + p]
        xt = np.ascontiguousarray(
            shard.T.reshape(ks // xg, xg, P, b_sh).transpose(0, 2, 1, 3))
        in_maps.append({"xt": xt, "wp": wp, "bvec": bvec})

    global _last_in_maps
    _last_in_maps = in_maps
    nc = _get_nc(make_key(1))
    res = run_bass_kernel_spmd(nc, in_maps, list(range(N_CORES)), trace=TRACE)
    LAST_RESULTS = res

    final = np.empty((batch, OUT_F), dtype=ml_dtypes.bfloat16)
    for c in range(N_CORES):
        oc = res.results[c]["out"].reshape(N_PAD, b_sh)
        final[c * b_sh:(c + 1) * b_sh, :] = oc[:OUT_F].T
    return final

